# revision 24
# baseline (speedup 1.0000x reference)
"""Multi-head attention (B=2, L=2048, D=2048, 16 heads of 128) on 8 NeuronCores.

Tensor-parallel over heads: each core projects q/k/v for its 2 heads
(weights pre-sliced + pre-transposed on host), runs RoPE + softmax attention
in a transposed [hd, pos] / [k, q] layout (no on-chip softmax-axis
transposes needed), applies the output projection against its slice of wo,
and returns a partial y.T. The host sums the 8 partials (the "all-reduce")
and adds the bias.

Matmuls run as float32r (TF32-like, full PE rate at N>=256). Softmax skips
the max-subtraction (scores are O(5) for unit-variance inputs); the
denominator comes from ones-column matmuls and is applied via a K=1
broadcast outer-product + elementwise multiply.
"""

import numpy as np
from contextlib import ExitStack

import concourse.bacc as bacc
import concourse.tile as tile
from concourse import mybir, masks
from concourse.bass_utils import run_bass_kernel_spmd

F32 = mybir.dt.float32
F32R = mybir.dt.float32r

B, L, D, H, HD = 2, 2048, 2048, 16, 128
NCORES = 8
HPC = H // NCORES  # heads per core
EC = HPC * HD  # per-core projection width (256)
ND = D // 128  # contraction tiles for the projections
NKT = L // 128  # k tiles per batch
QC = 512  # q/bl chunk
NQC = L // QC  # q chunks per batch
SCALE = float(HD) ** -0.5

_nc_cache = None


def _build():
    nc = bacc.Bacc()
    xt = nc.dram_tensor("xt", [D, B * L], F32R, kind="ExternalInput")
    wqt = nc.dram_tensor("wqt", [D, EC], F32R, kind="ExternalInput")
    wkt = nc.dram_tensor("wkt", [D, EC], F32R, kind="ExternalInput")
    wvt = nc.dram_tensor("wvt", [D, EC], F32R, kind="ExternalInput")
    wot = nc.dram_tensor("wot", [EC, D], F32R, kind="ExternalInput")
    cost = nc.dram_tensor("cost", [HD, L], F32, kind="ExternalInput")
    sinst = nc.dram_tensor("sinst", [HD, L], F32, kind="ExternalInput")
    yt = nc.dram_tensor("yt", [D, B * L], F32, kind="ExternalOutput")

    Exp = mybir.ActivationFunctionType.Exp

    with tile.TileContext(nc) as tc, ExitStack() as ctx:
        persist = ctx.enter_context(tc.tile_pool(name="persist", bufs=1))
        qk = ctx.enter_context(tc.tile_pool(name="qk", bufs=1))
        stream = ctx.enter_context(tc.tile_pool(name="stream", bufs=2))
        x_pool = ctx.enter_context(tc.tile_pool(name="x_pool", bufs=5))
        ex_pool = ctx.enter_context(tc.tile_pool(name="ex_pool", bufs=3))
        y_pool = ctx.enter_context(tc.tile_pool(name="y_pool", bufs=2))
        # psum: tags A,B = 2x single-bank slots; C,D = one 2-bank pair slot
        psp = ctx.enter_context(tc.tile_pool(name="psp", bufs=2, space="PSUM"))

        def ps1(tag, shape=(128, QC), name="ps", bufs=None):
            return psp.tile(list(shape), F32, tag=tag, name=f"{name}_{tag}", bufs=bufs)

        # constants
        idc = persist.tile([128, 128], F32)
        masks.make_identity(nc, idc[:])
        onescol_f = persist.tile([128, 1], F32)
        nc.vector.memset(onescol_f[:], 1.0)
        ones_k = persist.tile([128, 1], F32R)
        nc.vector.tensor_copy(ones_k[:], onescol_f[:])
        onesrow_f = persist.tile([1, 128], F32)
        nc.vector.memset(onesrow_f[:], 1.0)
        ones1 = persist.tile([1, 128], F32R)
        nc.vector.tensor_copy(ones1[:], onesrow_f[:])

        # weights: one tile per contraction d-tile so matmuls start as soon
        # as their slice lands
        w_sb = {}
        for wname, dr in (("q", wqt), ("k", wkt), ("v", wvt)):
            for t in range(ND):
                w_t = persist.tile([128, EC], F32R, tag=f"w{wname}{t}", name=f"w{wname}{t}")
                nc.sync.dma_start(w_t[:], dr[t * 128 : (t + 1) * 128, :])
                w_sb[wname, t] = w_t
        wo_sb = persist.tile([128, HPC * D], F32R)
        for h in range(HPC):
            nc.sync.dma_start(
                wo_sb[:, h * D : (h + 1) * D], wot[h * 128 : (h + 1) * 128, :]
            )
        cos_sb = persist.tile([128, L], F32)
        sin_sb = persist.tile([128, L], F32)
        nc.sync.dma_start(cos_sb[:], cost[:])
        nc.sync.dma_start(sin_sb[:], sinst[:])

        pend = None
        for b in range(B):
            # per-batch q/k (rope'd, [hd, l]) and v ([l, hd] 128-blocks packed)
            qT = [
                [qk.tile([128, QC], F32R, tag=f"qT{h}_{cc}", name=f"qT{h}_{cc}") for cc in range(NQC)]
                for h in range(HPC)
            ]
            kT = [
                [qk.tile([128, QC], F32R, tag=f"kT{h}_{cc}", name=f"kT{h}_{cc}") for cc in range(NQC)]
                for h in range(HPC)
            ]
            v_sb = [
                [qk.tile([128, QC], F32R, tag=f"v{h}_{cc}", name=f"v{h}_{cc}") for cc in range(NQC)]
                for h in range(HPC)
            ]

            # ---- projections ----
            for c in range(NQC):
                lsl = slice(c * QC, (c + 1) * QC)
                ps_qkv = {}
                for h in range(HPC):
                    ps_qkv["q", h] = ps1("A", name=f"pjq{h}")
                    ps_qkv["k", h] = ps1("B", name=f"pjk{h}")
                ps_v2 = ps1("C", (128, 2 * QC), name="pjv", bufs=1)
                for h in range(HPC):
                    ps_qkv["v", h] = ps_v2[:, h * QC : (h + 1) * QC]
                for t in range(ND):
                    x_t = x_pool.tile([128, QC], F32R, tag="x")
                    nc.sync.dma_start(
                        x_t[:],
                        xt[t * 128 : (t + 1) * 128, b * L + c * QC : b * L + (c + 1) * QC],
                    )
                    kw = dict(start=(t == 0), stop=(t == ND - 1))
                    for wn in ("q", "k", "v"):
                        for h in range(HPC):
                            nc.tensor.matmul(
                                ps_qkv[wn, h][:],
                                w_sb[wn, t][:, h * HD : (h + 1) * HD],
                                x_t[:],
                                **kw,
                            )
                # v: psum -> sbuf, transpose 128-blocks, pack [l, hd] per head
                vraw = stream.tile([128, 2 * QC], F32, tag="vraw")
                nc.scalar.copy(vraw[:], ps_v2[:])
                ps_t = ps1("D", (128, 2 * QC), name="ps_t", bufs=1)
                for h in range(HPC):
                    for j in range(QC // 128):
                        o = h * QC + j * 128
                        nc.tensor.transpose(
                            ps_t[:, o : o + 128], vraw[:, o : o + 128], idc[:]
                        )
                for h in range(HPC):
                    nc.vector.tensor_copy(v_sb[h][c][:], ps_t[:, h * QC : (h + 1) * QC])

                # rope on q and k; psum is released by the single ACT copy,
                # the muls run on sbuf
                for wn, dst_list in (("k", kT), ("q", qT)):
                    for h in range(HPC):
                        raw = stream.tile([128, QC], F32, tag="ropeA")
                        nc.scalar.copy(raw[:], ps_qkv[wn, h][:])  # psum release
                        shuf = stream.tile([128, QC], F32, tag="ropeB")
                        nc.gpsimd.dma_start(shuf[0:64, :], raw[64:128, :])
                        nc.gpsimd.dma_start(shuf[64:128, :], raw[0:64, :])
                        pA = stream.tile([128, QC], F32, tag="ropeC")
                        nc.vector.tensor_mul(pA[:], raw[:], cos_sb[:, lsl])
                        pB = stream.tile([128, QC], F32, tag="ropeD")
                        nc.vector.tensor_mul(pB[:], shuf[:], sin_sb[:, lsl])
                        nc.vector.tensor_add(dst_list[h][c][:], pA[:], pB[:])
            # ---- attention + output projection, per q chunk ----
            # oproj for chunk c-1 is emitted between chunk c's k-loops and
            # chunk c's norm chains, so the PE never waits on the softmax
            # denominator chain or on psum drains at chunk boundaries.
            def emit_oproj(pb, pc, pnorm):
                for e in range(ND):
                    ps_y = ps1("B", name="ps_y")
                    for h in range(HPC):
                        off = h * D + e * 128
                        nc.tensor.matmul(
                            ps_y[:],
                            wo_sb[:, off : off + 128],
                            pnorm[h][:],
                            start=(h == 0),
                            stop=(h == HPC - 1),
                        )
                    yst = y_pool.tile([128, QC], F32, tag="yst")
                    if e % 2 == 0:
                        nc.vector.tensor_copy(yst[:], ps_y[:])
                    else:
                        nc.scalar.copy(yst[:], ps_y[:])
                    nc.gpsimd.dma_start(
                        yt[
                            e * 128 : (e + 1) * 128,
                            pb * L + pc * QC : pb * L + (pc + 1) * QC,
                        ],
                        yst[:],
                    )

            for c in range(NQC):
                qsl = slice(c * QC, (c + 1) * QC)
                ps_o = []
                ps_sum = []
                rcprs = []
                for h in range(HPC):
                    ps_o.append(ps1("A", name="ps_o"))
                    ps_sum.append(ps1("B", (1, QC), name="ps_sum"))

                    def emit_pv_sums(tp, ex, h=h):
                        for j in range(2):
                            t = 2 * tp + j
                            kw = dict(start=(t == 0), stop=(t == NKT - 1))
                            nc.tensor.matmul(
                                ps_o[h][:],
                                v_sb[h][t // 4][:, (t % 4) * 128 : (t % 4 + 1) * 128],
                                ex[:, j * QC : (j + 1) * QC],
                                **kw,
                            )
                        for j in range(2):
                            t = 2 * tp + j
                            kw = dict(start=(t == 0), stop=(t == NKT - 1))
                            nc.tensor.matmul(
                                ps_sum[h][:], ones_k[:], ex[:, j * QC : (j + 1) * QC], **kw
                            )

                    # scores run one pair-step ahead of pv/sums so the PE
                    # never waits on the exp activation
                    prev_ex = None
                    for tp in range(NKT // 2):
                        ps_s = ps1(
                            "C" if tp % 2 == 0 else "D", (128, 2 * QC), name="ps_s", bufs=1
                        )
                        for j in range(2):
                            t = 2 * tp + j
                            nc.tensor.matmul(
                                ps_s[:, j * QC : (j + 1) * QC],
                                kT[h][t // 4][:, (t % 4) * 128 : (t % 4 + 1) * 128],
                                qT[h][c][:],
                                start=True,
                                stop=True,
                            )
                        ex = ex_pool.tile([128, 2 * QC], F32R, tag="exp")
                        nc.scalar.activation(ex[:], ps_s[:], Exp, scale=SCALE)
                        if prev_ex is not None:
                            emit_pv_sums(tp - 1, prev_ex)
                        prev_ex = ex
                    emit_pv_sums(NKT // 2 - 1, prev_ex)
                    # denominator reciprocal immediately (DVE is idle during
                    # the next head's k-loop)
                    rcp32 = stream.tile([1, QC], F32, tag="rcp32", bufs=1)
                    nc.vector.reciprocal_approx_fast(rcp32[:], ps_sum[h][:])
                    rcpr = stream.tile([1, QC], F32R, tag="rcpr", bufs=2)
                    nc.vector.tensor_copy(rcpr[:], rcp32[:])
                    rcprs.append(rcpr)
                if pend is not None:
                    emit_oproj(*pend)
                    pend = None
                norm = []
                for h in range(HPC):
                    ps_b = ps1("B", name="ps_b")
                    nc.tensor.matmul(ps_b[:], ones1[:], rcprs[h][:], start=True, stop=True)
                    bc = stream.tile([128, QC], F32, tag="bcast", bufs=1)
                    nc.vector.tensor_copy(bc[:], ps_b[:])
                    onorm = stream.tile([128, QC], F32R, tag=f"norm{h}")
                    nc.vector.tensor_mul(onorm[:], ps_o[h][:], bc[:])
                    norm.append(onorm)
                pend = (b, c, norm)
        if pend is not None:
            _b, _c, _norm = pend
            for e in range(ND):
                ps_y = ps1("B", name="ps_y")
                for h in range(HPC):
                    off = h * D + e * 128
                    nc.tensor.matmul(
                        ps_y[:],
                        wo_sb[:, off : off + 128],
                        _norm[h][:],
                        start=(h == 0),
                        stop=(h == HPC - 1),
                    )
                yst = y_pool.tile([128, QC], F32, tag="yst")
                nc.vector.tensor_copy(yst[:], ps_y[:])
                nc.gpsimd.dma_start(
                    yt[e * 128 : (e + 1) * 128, _b * L + _c * QC : _b * L + (_c + 1) * QC],
                    yst[:],
                )
    nc.finalize()
    return nc


def _get_nc():
    global _nc_cache
    if _nc_cache is None:
        _nc_cache = _build()
    return _nc_cache


def _prepare_in_maps(inputs):
    x = np.asarray(inputs["x"], np.float32)
    rope = np.asarray(inputs["rope_emb"], np.float32)
    wq = np.asarray(inputs["wq"], np.float32)
    wk = np.asarray(inputs["wk"], np.float32)
    wv = np.asarray(inputs["wv"], np.float32)
    wo = np.asarray(inputs["wo"], np.float32)

    xt = np.ascontiguousarray(x.reshape(B * L, D).T)
    cosT = np.ascontiguousarray(np.cos(rope).T)  # [HD, L]
    sinT = np.sin(rope).T  # [HD, L]
    sinsT = np.ascontiguousarray(np.concatenate([-sinT[: HD // 2], sinT[HD // 2 :]], 0))

    in_maps = []
    for c in range(NCORES):
        rows = slice(c * EC, (c + 1) * EC)
        in_maps.append(
            {
                "xt": xt,
                "cost": cosT,
                "sinst": sinsT,
                "wqt": np.ascontiguousarray(wq[rows].T),
                "wkt": np.ascontiguousarray(wk[rows].T),
                "wvt": np.ascontiguousarray(wv[rows].T),
                "wot": np.ascontiguousarray(wo[:, rows].T),
            }
        )
    return in_maps


def kernel(**inputs):
    bo = np.asarray(inputs["bo"], np.float32)
    in_maps = _prepare_in_maps(inputs)
    nc = _get_nc()
    res = run_bass_kernel_spmd(nc, in_maps, core_ids=list(range(NCORES)))
    y_t = res.results[0]["yt"]
    for c in range(1, NCORES):
        y_t = y_t + res.results[c]["yt"]
    y = y_t.T.reshape(B, L, D) + bo[None, None, :]
    return y.astype(np.float32)


# revision 25
# speedup vs baseline: 1.0025x; 1.0025x over previous
"""Multi-head attention (B=2, L=2048, D=2048, 16 heads of 128) on 8 NeuronCores.

Tensor-parallel over heads: each core projects q/k/v for its 2 heads
(weights pre-sliced + pre-transposed on host), runs RoPE + softmax attention
in a transposed [hd, pos] / [k, q] layout (no on-chip softmax-axis
transposes needed), applies the output projection against its slice of wo,
and returns a partial y.T. The host sums the 8 partials (the "all-reduce")
and adds the bias.

Matmuls run as float32r (TF32-like, full PE rate at N>=256). Softmax skips
the max-subtraction (scores are O(5) for unit-variance inputs); the
denominator comes from ones-column matmuls and is applied via a K=1
broadcast outer-product + elementwise multiply.
"""

import numpy as np
from contextlib import ExitStack

import concourse.bacc as bacc
import concourse.tile as tile
from concourse import mybir, masks
from concourse.bass_utils import run_bass_kernel_spmd

F32 = mybir.dt.float32
F32R = mybir.dt.float32r

B, L, D, H, HD = 2, 2048, 2048, 16, 128
NCORES = 8
HPC = H // NCORES  # heads per core
EC = HPC * HD  # per-core projection width (256)
ND = D // 128  # contraction tiles for the projections
NKT = L // 128  # k tiles per batch
QC = 512  # q/bl chunk
NQC = L // QC  # q chunks per batch
SCALE = float(HD) ** -0.5

_nc_cache = None


def _build():
    nc = bacc.Bacc()
    xt = nc.dram_tensor("xt", [D, B * L], F32R, kind="ExternalInput")
    wqt = nc.dram_tensor("wqt", [D, EC], F32R, kind="ExternalInput")
    wkt = nc.dram_tensor("wkt", [D, EC], F32R, kind="ExternalInput")
    wvt = nc.dram_tensor("wvt", [D, EC], F32R, kind="ExternalInput")
    wot = nc.dram_tensor("wot", [EC, D], F32R, kind="ExternalInput")
    cost = nc.dram_tensor("cost", [HD, L], F32, kind="ExternalInput")
    sinst = nc.dram_tensor("sinst", [HD, L], F32, kind="ExternalInput")
    yt = nc.dram_tensor("yt", [D, B * L], F32, kind="ExternalOutput")

    Exp = mybir.ActivationFunctionType.Exp

    with tile.TileContext(nc) as tc, ExitStack() as ctx:
        persist = ctx.enter_context(tc.tile_pool(name="persist", bufs=1))
        qk = ctx.enter_context(tc.tile_pool(name="qk", bufs=1))
        stream = ctx.enter_context(tc.tile_pool(name="stream", bufs=2))
        x_pool = ctx.enter_context(tc.tile_pool(name="x_pool", bufs=5))
        ex_pool = ctx.enter_context(tc.tile_pool(name="ex_pool", bufs=3))
        y_pool = ctx.enter_context(tc.tile_pool(name="y_pool", bufs=2))
        # psum: tags A,B = 2x single-bank slots; C,D = one 2-bank pair slot
        psp = ctx.enter_context(tc.tile_pool(name="psp", bufs=2, space="PSUM"))

        def ps1(tag, shape=(128, QC), name="ps", bufs=None):
            return psp.tile(list(shape), F32, tag=tag, name=f"{name}_{tag}", bufs=bufs)

        # constants
        idc = persist.tile([128, 128], F32)
        masks.make_identity(nc, idc[:])
        onescol_f = persist.tile([128, 1], F32)
        nc.vector.memset(onescol_f[:], 1.0)
        ones_k = persist.tile([128, 1], F32R)
        nc.vector.tensor_copy(ones_k[:], onescol_f[:])
        onesrow_f = persist.tile([1, 128], F32)
        nc.vector.memset(onesrow_f[:], 1.0)
        ones1 = persist.tile([1, 128], F32R)
        nc.vector.tensor_copy(ones1[:], onesrow_f[:])

        # weights: one tile per contraction d-tile so matmuls start as soon
        # as their slice lands
        w_sb = {}
        for wname, dr in (("q", wqt), ("k", wkt), ("v", wvt)):
            for t in range(ND):
                w_t = persist.tile([128, EC], F32R, tag=f"w{wname}{t}", name=f"w{wname}{t}")
                nc.sync.dma_start(w_t[:], dr[t * 128 : (t + 1) * 128, :])
                w_sb[wname, t] = w_t
        wo_sb = persist.tile([128, HPC * D], F32R)
        for h in range(HPC):
            nc.sync.dma_start(
                wo_sb[:, h * D : (h + 1) * D], wot[h * 128 : (h + 1) * 128, :]
            )
        cos_sb = persist.tile([128, L], F32)
        sin_sb = persist.tile([128, L], F32)
        nc.sync.dma_start(cos_sb[:], cost[:])
        nc.sync.dma_start(sin_sb[:], sinst[:])

        pend = None
        for b in range(B):
            # per-batch q/k (rope'd, [hd, l]) and v ([l, hd] 128-blocks packed)
            qT = [
                [qk.tile([128, QC], F32R, tag=f"qT{h}_{cc}", name=f"qT{h}_{cc}") for cc in range(NQC)]
                for h in range(HPC)
            ]
            kT = [
                [qk.tile([128, QC], F32R, tag=f"kT{h}_{cc}", name=f"kT{h}_{cc}") for cc in range(NQC)]
                for h in range(HPC)
            ]
            v_sb = [
                [qk.tile([128, QC], F32R, tag=f"v{h}_{cc}", name=f"v{h}_{cc}") for cc in range(NQC)]
                for h in range(HPC)
            ]

            # ---- projections ----
            for c in range(NQC):
                lsl = slice(c * QC, (c + 1) * QC)
                ps_qkv = {}
                for h in range(HPC):
                    ps_qkv["q", h] = ps1("A", name=f"pjq{h}")
                    ps_qkv["k", h] = ps1("B", name=f"pjk{h}")
                ps_v2 = ps1("C", (128, 2 * QC), name="pjv", bufs=1)
                for h in range(HPC):
                    ps_qkv["v", h] = ps_v2[:, h * QC : (h + 1) * QC]
                for t in range(ND):
                    x_t = x_pool.tile([128, QC], F32R, tag="x")
                    eng = nc.sync if t % 2 == 0 else nc.gpsimd
                    eng.dma_start(
                        x_t[:],
                        xt[t * 128 : (t + 1) * 128, b * L + c * QC : b * L + (c + 1) * QC],
                    )
                    kw = dict(start=(t == 0), stop=(t == ND - 1))
                    for wn in ("q", "k", "v"):
                        for h in range(HPC):
                            nc.tensor.matmul(
                                ps_qkv[wn, h][:],
                                w_sb[wn, t][:, h * HD : (h + 1) * HD],
                                x_t[:],
                                **kw,
                            )
                # v: psum -> sbuf, transpose 128-blocks, pack [l, hd] per head
                vraw = stream.tile([128, 2 * QC], F32, tag="vraw")
                nc.scalar.copy(vraw[:], ps_v2[:])
                ps_t = ps1("D", (128, 2 * QC), name="ps_t", bufs=1)
                for h in range(HPC):
                    for j in range(QC // 128):
                        o = h * QC + j * 128
                        nc.tensor.transpose(
                            ps_t[:, o : o + 128], vraw[:, o : o + 128], idc[:]
                        )
                for h in range(HPC):
                    nc.vector.tensor_copy(v_sb[h][c][:], ps_t[:, h * QC : (h + 1) * QC])

                # rope on q and k; psum is released by the single ACT copy,
                # the muls run on sbuf
                for wn, dst_list in (("k", kT), ("q", qT)):
                    for h in range(HPC):
                        raw = stream.tile([128, QC], F32, tag="ropeA")
                        nc.scalar.copy(raw[:], ps_qkv[wn, h][:])  # psum release
                        shuf = stream.tile([128, QC], F32, tag="ropeB")
                        nc.gpsimd.dma_start(shuf[0:64, :], raw[64:128, :])
                        nc.gpsimd.dma_start(shuf[64:128, :], raw[0:64, :])
                        pA = stream.tile([128, QC], F32, tag="ropeC")
                        nc.vector.tensor_mul(pA[:], raw[:], cos_sb[:, lsl])
                        pB = stream.tile([128, QC], F32, tag="ropeD")
                        nc.vector.tensor_mul(pB[:], shuf[:], sin_sb[:, lsl])
                        nc.vector.tensor_add(dst_list[h][c][:], pA[:], pB[:])
            # ---- attention + output projection, per q chunk ----
            # oproj for chunk c-1 is emitted between chunk c's k-loops and
            # chunk c's norm chains, so the PE never waits on the softmax
            # denominator chain or on psum drains at chunk boundaries.
            def emit_oproj(pb, pc, pnorm):
                for e in range(ND):
                    ps_y = ps1("B", name="ps_y")
                    for h in range(HPC):
                        off = h * D + e * 128
                        nc.tensor.matmul(
                            ps_y[:],
                            wo_sb[:, off : off + 128],
                            pnorm[h][:],
                            start=(h == 0),
                            stop=(h == HPC - 1),
                        )
                    yst = y_pool.tile([128, QC], F32, tag="yst")
                    if e % 2 == 0:
                        nc.vector.tensor_copy(yst[:], ps_y[:])
                    else:
                        nc.scalar.copy(yst[:], ps_y[:])
                    nc.sync.dma_start(
                        yt[
                            e * 128 : (e + 1) * 128,
                            pb * L + pc * QC : pb * L + (pc + 1) * QC,
                        ],
                        yst[:],
                    )

            for c in range(NQC):
                qsl = slice(c * QC, (c + 1) * QC)
                ps_o = []
                ps_sum = []
                rcprs = []
                for h in range(HPC):
                    ps_o.append(ps1("A", name="ps_o"))
                    ps_sum.append(ps1("B", (1, QC), name="ps_sum"))

                    def emit_pv_sums(tp, ex, h=h):
                        for j in range(2):
                            t = 2 * tp + j
                            kw = dict(start=(t == 0), stop=(t == NKT - 1))
                            nc.tensor.matmul(
                                ps_o[h][:],
                                v_sb[h][t // 4][:, (t % 4) * 128 : (t % 4 + 1) * 128],
                                ex[:, j * QC : (j + 1) * QC],
                                **kw,
                            )
                        for j in range(2):
                            t = 2 * tp + j
                            kw = dict(start=(t == 0), stop=(t == NKT - 1))
                            nc.tensor.matmul(
                                ps_sum[h][:], ones_k[:], ex[:, j * QC : (j + 1) * QC], **kw
                            )

                    # scores run one pair-step ahead of pv/sums so the PE
                    # never waits on the exp activation
                    prev_ex = None
                    for tp in range(NKT // 2):
                        ps_s = ps1(
                            "C" if tp % 2 == 0 else "D", (128, 2 * QC), name="ps_s", bufs=1
                        )
                        for j in range(2):
                            t = 2 * tp + j
                            nc.tensor.matmul(
                                ps_s[:, j * QC : (j + 1) * QC],
                                kT[h][t // 4][:, (t % 4) * 128 : (t % 4 + 1) * 128],
                                qT[h][c][:],
                                start=True,
                                stop=True,
                            )
                        ex = ex_pool.tile([128, 2 * QC], F32R, tag="exp")
                        nc.scalar.activation(ex[:], ps_s[:], Exp, scale=SCALE)
                        if prev_ex is not None:
                            emit_pv_sums(tp - 1, prev_ex)
                        prev_ex = ex
                    emit_pv_sums(NKT // 2 - 1, prev_ex)
                    # denominator reciprocal immediately (DVE is idle during
                    # the next head's k-loop)
                    rcp32 = stream.tile([1, QC], F32, tag="rcp32", bufs=1)
                    nc.vector.reciprocal_approx_fast(rcp32[:], ps_sum[h][:])
                    rcpr = stream.tile([1, QC], F32R, tag="rcpr", bufs=2)
                    nc.vector.tensor_copy(rcpr[:], rcp32[:])
                    rcprs.append(rcpr)
                if pend is not None:
                    emit_oproj(*pend)
                    pend = None
                norm = []
                for h in range(HPC):
                    ps_b = ps1("B", name="ps_b")
                    nc.tensor.matmul(ps_b[:], ones1[:], rcprs[h][:], start=True, stop=True)
                    bc = stream.tile([128, QC], F32, tag="bcast", bufs=1)
                    nc.vector.tensor_copy(bc[:], ps_b[:])
                    onorm = stream.tile([128, QC], F32R, tag=f"norm{h}")
                    nc.vector.tensor_mul(onorm[:], ps_o[h][:], bc[:])
                    norm.append(onorm)
                pend = (b, c, norm)
        if pend is not None:
            _b, _c, _norm = pend
            for e in range(ND):
                ps_y = ps1("B", name="ps_y")
                for h in range(HPC):
                    off = h * D + e * 128
                    nc.tensor.matmul(
                        ps_y[:],
                        wo_sb[:, off : off + 128],
                        _norm[h][:],
                        start=(h == 0),
                        stop=(h == HPC - 1),
                    )
                yst = y_pool.tile([128, QC], F32, tag="yst")
                nc.vector.tensor_copy(yst[:], ps_y[:])
                nc.sync.dma_start(
                    yt[e * 128 : (e + 1) * 128, _b * L + _c * QC : _b * L + (_c + 1) * QC],
                    yst[:],
                )
    nc.finalize()
    return nc


def _get_nc():
    global _nc_cache
    if _nc_cache is None:
        _nc_cache = _build()
    return _nc_cache


def _prepare_in_maps(inputs):
    x = np.asarray(inputs["x"], np.float32)
    rope = np.asarray(inputs["rope_emb"], np.float32)
    wq = np.asarray(inputs["wq"], np.float32)
    wk = np.asarray(inputs["wk"], np.float32)
    wv = np.asarray(inputs["wv"], np.float32)
    wo = np.asarray(inputs["wo"], np.float32)

    xt = np.ascontiguousarray(x.reshape(B * L, D).T)
    cosT = np.ascontiguousarray(np.cos(rope).T)  # [HD, L]
    sinT = np.sin(rope).T  # [HD, L]
    sinsT = np.ascontiguousarray(np.concatenate([-sinT[: HD // 2], sinT[HD // 2 :]], 0))

    in_maps = []
    for c in range(NCORES):
        rows = slice(c * EC, (c + 1) * EC)
        in_maps.append(
            {
                "xt": xt,
                "cost": cosT,
                "sinst": sinsT,
                "wqt": np.ascontiguousarray(wq[rows].T),
                "wkt": np.ascontiguousarray(wk[rows].T),
                "wvt": np.ascontiguousarray(wv[rows].T),
                "wot": np.ascontiguousarray(wo[:, rows].T),
            }
        )
    return in_maps


def kernel(**inputs):
    bo = np.asarray(inputs["bo"], np.float32)
    in_maps = _prepare_in_maps(inputs)
    nc = _get_nc()
    res = run_bass_kernel_spmd(nc, in_maps, core_ids=list(range(NCORES)))
    y_t = res.results[0]["yt"]
    for c in range(1, NCORES):
        y_t = y_t + res.results[c]["yt"]
    y = y_t.T.reshape(B, L, D) + bo[None, None, :]
    return y.astype(np.float32)
